# revision 1
# baseline (speedup 1.0000x reference)
"""Trainium2 Bass kernel for a continuous-convolution layer.

Math: out[b,x,c] = (1/S) * sum_s f(||x_bx - y_bs||^2) * u[b,s,c]
where f is a fixed scalar->scalar MLP (width 16, depth 4, tanh residual
blocks) whose weights arrive as runtime inputs.

Strategy:
  * Every one of the B*S*X = 4.2M points evaluates the SAME scalar
    function f(r).  On the host we fit f with an affine term plus a small
    sum of decaying exponentials (Prony-type fit; the tanh-saturation
    corrections of f decay exponentially in r, so an exp basis converges
    fast).  Amplitudes come from a weighted ridge least-squares solve on a
    dense grid; exponents start from a tuned set and are optionally
    refined with scipy's variable-projection least squares.
  * On each NeuronCore (sensor-sharded: 128 sensors/core, all batches,
    all eval points):
      - r is produced by a K=4 fp32 matmul:  r = ly^T @ lx with
        ly = [y0, y1, |y|^2, 1], lx = [-2*x0, -2*x1, 1, |x|^2]  -> PSUM
      - the affine term (c1*r + c0) is one ACT Copy op (scale/bias
        immediates); each exp unit is one ACT Exp op (scale=-s_j), output
        cast to bf16 for a fast einsum.
      - the sensor reduction folds the basis amplitudes into the sensor
        values: out[c,x] = sum_j sum_s (a_j*u[s,c]) * t_j[s,x], which is
        a chain of matmuls accumulating in PSUM (fp32 for the affine
        term, bf16 for the bounded exp units).
  * Host sums the per-core partials (the sensor shards) and transposes.
"""

import numpy as np
import ml_dtypes

B, S, X, C, D = 4, 1024, 1024, 8, 2
WIDTH, DEPTH = 16, 4
N_CORES = 8
SPC = S // N_CORES  # sensors per core = 128

# Exponents tuned offline for this problem family (rmax ~ 64); rescaled at
# runtime by the actual r range and optionally refined with scipy.
S_INIT_6 = np.array([0.135, 0.232, 0.363, 1.703, 12.149, 16.401])
S_INIT_8 = np.array([0.063, 0.430, 0.749, 1.363, 2.460, 4.447, 7.836, 11.720])
S_INIT_12 = np.array([0.021, 0.035, 0.078, 0.119, 0.182, 0.368, 0.725,
                      1.350, 2.453, 4.447, 7.842, 11.729])
RMAX_REF = 63.85


def _f_exact(r, W_in, b_in, W_hid, b_hid, W_out, b_out):
    """Reference scalar function f(r), evaluated in float64 on the host."""
    h = r[:, None] * W_in[0].astype(np.float64) + b_in.astype(np.float64)
    for l in range(DEPTH):
        h = np.tanh(h @ W_hid[l].astype(np.float64)
                    + b_hid[l].astype(np.float64)) + h
    return (h @ W_out.astype(np.float64))[:, 0] + np.float64(b_out[0])


def _fit_exp_basis(r_flat, f_args, s_init):
    """Fit f(r) ~ c0 + c1*r + sum_j a_j exp(-s_j r) on [0, rmax].

    Returns (c0, c1, s, a, quality) where quality is the density-weighted
    rms residual (a proxy for the end-to-end output error).
    """
    rmax = float(r_flat.max()) * 1.02
    grid = np.unique(np.concatenate([
        np.linspace(0.0, min(8.0, rmax), 6000),
        np.linspace(min(8.0, rmax), rmax, 6000)]))
    fg = _f_exact(grid, *f_args)

    hist, edges = np.histogram(r_flat, bins=512, range=(0.0, rmax))
    dens = hist / max(1, hist.max())
    gd = np.interp(grid, 0.5 * (edges[1:] + edges[:-1]), dens)
    wgt = np.sqrt(gd + 1e-3)
    wgt /= wgt.mean()

    aff = np.stack([np.ones_like(grid), grid], 1)
    lam = 3e-4

    def solve_amp(s):
        A = np.concatenate([aff, np.exp(-np.outer(grid, s))], 1)
        Aw = A * wgt[:, None]
        n = A.shape[1]
        G = Aw.T @ Aw + lam * np.diag(np.r_[0.0, 0.0, np.ones(n - 2)])
        c = np.linalg.solve(G, Aw.T @ (fg * wgt))
        res = (fg - A @ c) * wgt
        return c, res

    s = np.asarray(s_init, dtype=np.float64) * (RMAX_REF / rmax)
    c, res = solve_amp(s)
    try:  # optional nonlinear refinement of the exponents
        from scipy.optimize import least_squares

        def proj(logs):
            cc, rr = solve_amp(np.exp(logs))
            return np.r_[rr, np.sqrt(lam) * cc[2:]]

        sol = least_squares(proj, np.log(s), method='lm', max_nfev=1500)
        s_ref = np.exp(sol.x)
        c_ref, res_ref = solve_amp(s_ref)
        if np.sqrt(np.mean(res_ref ** 2)) < np.sqrt(np.mean(res ** 2)):
            s, c, res = s_ref, c_ref, res_ref
    except Exception:
        pass
    quality = float(np.sqrt(np.mean(res ** 2)))
    return float(c[0]), float(c[1]), s, c[2:], quality


def _build_and_run(ly_np, lx_np, uw_np, s_exp, n_units):
    """Build the Bass program (W = n_units exp units) and run it on 8 cores."""
    import concourse.bass as bass
    import concourse.mybir as mybir
    from concourse.bass_utils import run_bass_kernel_spmd

    AF = mybir.ActivationFunctionType
    W = n_units

    nc = bass.Bass()
    ly_d = nc.declare_dram_parameter("ly", [4, B * SPC], mybir.dt.float32, isOutput=False)
    lx_d = nc.declare_dram_parameter("lx", [4, B * X], mybir.dt.float32, isOutput=False)
    uw_d = nc.declare_dram_parameter("uw", [SPC, B * W * C], mybir.dt.bfloat16, isOutput=False)
    o_d = nc.declare_dram_parameter("o", [B, C, X], mybir.dt.float32, isOutput=True)

    from contextlib import ExitStack

    with ExitStack() as ctx:
        ly = ctx.enter_context(nc.sbuf_tensor([4, B * SPC], mybir.dt.float32))
        lx = ctx.enter_context(nc.sbuf_tensor([4, B * X], mybir.dt.float32))
        uw = ctx.enter_context(nc.sbuf_tensor([SPC, B * W * C], mybir.dt.bfloat16))
        prs = [ctx.enter_context(nc.psum_tensor(f"pr{i}", [SPC, X], mybir.dt.float32)) for i in range(2)]
        pos = [ctx.enter_context(nc.psum_tensor(f"po{i}", [C, X], mybir.dt.float32)) for i in range(2)]
        tes = [ctx.enter_context(nc.sbuf_tensor(f"te{i}", [SPC, X], mybir.dt.bfloat16)) for i in range(4)]
        obs = [ctx.enter_context(nc.sbuf_tensor(f"ob{i}", [C, X], mybir.dt.float32)) for i in range(2)]
        s_ly = ctx.enter_context(nc.semaphore("s_ly"))
        s_lx = ctx.enter_context(nc.semaphore("s_lx"))
        s_r = ctx.enter_context(nc.semaphore("s_r"))
        s_act = ctx.enter_context(nc.semaphore("s_act"))
        s_ein = ctx.enter_context(nc.semaphore("s_ein"))
        s_dve = ctx.enter_context(nc.semaphore("s_dve"))
        s_out = ctx.enter_context(nc.semaphore("s_out"))
        s_uw = ctx.enter_context(nc.semaphore("s_uw"))
        block = ctx.enter_context(nc.Block())

        @block.sync
        def _(sync):
            # one semaphore per input tensor: same-engine DMAs may complete
            # out of order across HW-DGE queues, so a shared counter cannot
            # tell WHICH transfer finished.
            sync.dma_start(out=ly[:], in_=ly_d[:]).then_inc(s_ly, 16)
            sync.dma_start(out=lx[:], in_=lx_d[:]).then_inc(s_lx, 16)
            for b in range(B):
                sync.wait_ge(s_dve, b + 1)
                sync.dma_start(out=o_d[b], in_=obs[b % 2][:]).then_inc(s_out, 16)

        @block.gpsimd
        def _(gpsimd):
            gpsimd.dma_start(out=uw[:], in_=uw_d[:]).then_inc(s_uw, 16)

        @block.tensor
        def _(tensor):
            def emit_r(b):
                if b >= 2:
                    # psum_r[b%2] is free once every ACT of batch b-2 read it
                    tensor.wait_ge(s_act, (b - 1) * W)
                pr = prs[b % 2]
                for h in range(2):
                    mm = tensor.matmul(
                        pr[:, 512 * h:512 * (h + 1)],
                        ly[:, SPC * b:SPC * (b + 1)],
                        lx[:, X * b + 512 * h:X * b + 512 * (h + 1)],
                        start=True, stop=True)
                    if h == 1:
                        mm.then_inc(s_r, 1)

            def emit_ein(b):
                po = pos[b % 2]
                for j in range(W):
                    e = b * W + j
                    tensor.wait_ge(s_act, e + 1)
                    if b >= 2 and j == 0:
                        tensor.wait_ge(s_dve, b - 1)  # psum_o[b%2] drained
                    lhsT = uw[:, C * e:C * (e + 1)]
                    rhs = tes[e % 4]
                    for h in range(2):
                        mm = tensor.matmul(
                            po[:, 512 * h:512 * (h + 1)],
                            lhsT,
                            rhs[:, 512 * h:512 * (h + 1)],
                            start=(j == 0), stop=(j == W - 1))
                        if h == 1:
                            mm.then_inc(s_ein, 1)

            # software-pipelined: batch b+1's r-matmuls are issued before
            # batch b's einsums so ACT never waits at a batch boundary.
            tensor.wait_ge(s_ly, 16)
            tensor.wait_ge(s_lx, 16)
            emit_r(0)
            tensor.wait_ge(s_uw, 16)  # uw resident
            for b in range(1, B):
                emit_r(b)
                emit_ein(b - 1)
            emit_ein(B - 1)

        @block.scalar
        def _(scalar):
            for b in range(B):
                scalar.wait_ge(s_r, b + 1)
                pr = prs[b % 2]
                for j in range(W):
                    e = b * W + j
                    if e >= 4:  # te[e%4] free once unit e-4's mms ran
                        scalar.wait_ge(s_ein, e - 3)
                    scalar.activation(tes[e % 4][:], pr[:], AF.Exp,
                                      scale=float(-s_exp[j])
                                      ).then_inc(s_act, 1)

        @block.vector
        def _(vector):
            for b in range(B):
                vector.wait_ge(s_ein, (b + 1) * W)
                if b >= 2:
                    vector.wait_ge(s_out, 16 * (b - 1))  # ob[b%2] stored
                vector.tensor_copy(obs[b % 2][:], pos[b % 2][:]).then_inc(s_dve, 1)

    in_maps = []
    for core in range(N_CORES):
        in_maps.append({
            "ly": ly_np[core], "lx": lx_np, "uw": uw_np[core],
        })
    res = run_bass_kernel_spmd(nc, in_maps, list(range(N_CORES)))
    global LAST_RESULT
    LAST_RESULT = res
    return res


LAST_RESULT = None


def kernel(yu, x, W_in, b_in, W_hid, b_hid, W_out, b_out):
    yu = np.asarray(yu, dtype=np.float32)
    x = np.asarray(x, dtype=np.float32)
    f_args = (np.asarray(W_in, np.float32), np.asarray(b_in, np.float32),
              np.asarray(W_hid, np.float32), np.asarray(b_hid, np.float32),
              np.asarray(W_out, np.float32), np.asarray(b_out, np.float32))

    y = yu[:, :, -D:].astype(np.float64)   # (B,S,2) sensor positions
    u = yu[:, :, :C].astype(np.float64)    # (B,S,C) sensor values
    xd = x.astype(np.float64)              # (B,X,2)

    # exact r for range/density statistics (cheap: ~4M values)
    r_all = (xd[:, None, :, 0] - y[:, :, None, 0]) ** 2 \
        + (xd[:, None, :, 1] - y[:, :, None, 1]) ** 2          # (B,S,X)
    r_flat = r_all.ravel()

    c0, c1, s_exp, amps, q = _fit_exp_basis(r_flat, f_args, S_INIT_6)
    for s_next in (S_INIT_8, S_INIT_12):  # guarded fallback: denser basis
        if q <= 2.5e-3:
            break
        c0b, c1b, s_b, a_b, qb = _fit_exp_basis(r_flat, f_args, s_next)
        if qb < q:
            c0, c1, s_exp, amps, q = c0b, c1b, s_b, a_b, qb
    n_units = len(s_exp)

    # ---- host-side packing (per core: a 128-sensor shard) ----
    ly_np = np.empty((N_CORES, 4, B * SPC), np.float32)
    uw_np = np.empty((N_CORES, SPC, B * n_units * C), ml_dtypes.bfloat16)
    for core in range(N_CORES):
        sl = slice(core * SPC, (core + 1) * SPC)
        for b in range(B):
            yb = y[b, sl]                                    # (128,2)
            ly_np[core, 0, SPC * b:SPC * (b + 1)] = yb[:, 0]
            ly_np[core, 1, SPC * b:SPC * (b + 1)] = yb[:, 1]
            ly_np[core, 2, SPC * b:SPC * (b + 1)] = (yb ** 2).sum(1)
            ly_np[core, 3, SPC * b:SPC * (b + 1)] = 1.0
            ub = u[b, sl]                                    # (128,C)
            for j in range(n_units):
                e = b * n_units + j
                uw_np[core, :, C * e:C * (e + 1)] = \
                    (amps[j] * ub).astype(np.float32)
    lx_np = np.empty((4, B * X), np.float32)
    for b in range(B):
        xb = xd[b]
        lx_np[0, X * b:X * (b + 1)] = -2.0 * xb[:, 0]
        lx_np[1, X * b:X * (b + 1)] = -2.0 * xb[:, 1]
        lx_np[2, X * b:X * (b + 1)] = 1.0
        lx_np[3, X * b:X * (b + 1)] = (xb ** 2).sum(1)

    res = _build_and_run(ly_np, lx_np, uw_np, s_exp, n_units)

    # ---- host-side unshard: sum sensor shards, transpose (b,c,x)->(b,x,c)
    acc = np.zeros((B, C, X), np.float64)
    for core in range(N_CORES):
        acc += res.results[core]["o"].astype(np.float64)
    out = acc.transpose(0, 2, 1) / S

    # exact affine contribution (1/S)*sum_s u[b,s,c]*(c0 + c1*r[b,s,x]):
    # r = |x|^2 - 2 x.y + |y|^2 gives a rank-2 structure over sensors.
    su = u.sum(1)                                   # (B,C)
    sur2 = np.einsum('bsc,bs->bc', u, (y ** 2).sum(-1))
    suy = np.einsum('bsc,bsd->bcd', u, y)           # (B,C,2)
    x2 = (xd ** 2).sum(-1)                          # (B,X)
    aff = (c0 * su[:, None, :]
           + c1 * (x2[:, :, None] * su[:, None, :]
                   + sur2[:, None, :]
                   - 2.0 * np.einsum('bxd,bcd->bxc', xd, suy))) / S
    return (out + aff).astype(np.float32)

